# revision 41
# baseline (speedup 1.0000x reference)
"""Trainium2 Bass kernel for nn_ExpertsMLPBlock (MoE routing).

Problem (hardcoded):
  x          [8, 4096, 256] f32
  rms_weight [256]          f32
  W1         [8, 256, 1024] f32
  b1         [8, 1024]      f32
  W2         [8, 1024, 256] f32
  b2         [8, 256]       f32
  expert_ids [8, 4096, 2]   int   (values 0..7)
  out        [8, 4096, 2, 256] f32

Sharding: EXPERT-parallel across the 8 NeuronCores (the spec's suggested
"shard W1/b1/W2/b2 along the expert axis and all-to-all tokens by
expert_id").  The all-to-all happens at shard/unshard time on the host:
core e receives exactly the token rows routed to expert e (deduplicated
when both k-slots of a token pick the same expert, padded to a common
block count so all cores run the same SPMD program), plus only expert
e's weights.  The core does all the value math on device:

  rstd = rsqrt(mean(x^2) + eps)            (vector square+reduce+bit-hack
                                            rsqrt with one Newton step)
  xn^T = (x_blk @ diag(rstd))^T [* rms_w]  (PE transpose fused with the
                                            rstd column scale; all-ones
                                            rms_weight multiply skipped)
  h^T  = gelu(W1^T xn^T + b1)              (PE matmul, scalar gelu)
  y    = h W2 + b2                         (PE matmul, vector bias add)

y rows come back rank-ordered per expert (bf16); the host places them
into the full [B,T,K,C] f32 output during unsharding.

Engine budget per core: PE is the bottleneck (~140us of matmul incl.
transposes); scalar runs gelu + half the transpose evacuations, vector
runs the rest.  (An XBAR dma-transpose variant was tried and regressed:
the DRAM bounce serializes the sync queue.)  GpSimd/Pool cannot touch
PSUM and its ALU ops are Q7-emulated, so it is left idle.
"""

import math

import numpy as np

import concourse.bacc as bacc
import concourse.bass as bass
import concourse.mybir as mybir
from concourse import bass_utils
from concourse.tile import TileContext
from concourse.alu_op_type import AluOpType

F32 = mybir.dt.float32
BF16 = mybir.dt.bfloat16
I32 = mybir.dt.int32

B, T, C, WH, E, K = 8, 4096, 256, 1024, 8, 2
NTOK = B * T            # 32768 tokens total
NSLOT = NTOK * K        # 65536 (token, k) slots
G = 4                   # blocks (of 128 rows) per pipeline group
RMS_EPS = 1.1920928955078125e-07
ACT_GELU = mybir.ActivationFunctionType.Gelu
ACT_COPY = mybir.ActivationFunctionType.Copy

_CACHE = {}


def _build(nb, apply_rmsw=False):
    """Per-core program: one expert's 2-layer MLP over nb*128 routed rows."""
    cap = nb * 128
    nc = bacc.Bacc("TRN2", target_bir_lowering=False, debug=False, num_devices=8)

    xg_d = nc.dram_tensor("xg", [cap, C], BF16, kind="ExternalInput")
    w1_d = nc.dram_tensor("w1", [128, 2, WH], BF16, kind="ExternalInput")
    w2_d = nc.dram_tensor("w2", [128, 8, C], BF16, kind="ExternalInput")
    b1_d = nc.dram_tensor("b1s", [128, 8], F32, kind="ExternalInput")
    b2_d = nc.dram_tensor("b2r", [128, C], F32, kind="ExternalInput")
    rmsw_d = nc.dram_tensor("rmsw2", [128, 2], F32, kind="ExternalInput")
    ident_d = nc.dram_tensor("identf", [128, 128], F32, kind="ExternalInput")
    y_d = nc.dram_tensor("y", [cap, C], BF16, kind="ExternalOutput")

    xg_pc = xg_d.ap().rearrange("(blk p) c -> p blk c", p=128)
    y_pc = y_d.ap().rearrange("(blk p) c -> p blk c", p=128)

    # ramp-up/ramp-down group schedule: small groups at both ends shorten
    # the pipeline fill (first matmul sooner) and drain (last store smaller)
    sizes = []
    rem = nb
    for s in (1, 2):
        if rem > s + 3:
            sizes.append(s)
            rem -= s
    tailsz = [1, 1] if rem > G + 2 else []
    rem -= len(tailsz)
    sizes += [G] * (rem // G)
    if rem % G:
        sizes.append(rem % G)
    sizes += tailsz
    groups = []
    g0 = 0
    for gs in sizes:
        groups.append((g0, gs))
        g0 += gs
    assert g0 == nb

    with TileContext(nc) as tc:
        with (
            tc.tile_pool(name="const", bufs=1) as constp,
            tc.tile_pool(name="xp", bufs=4) as xp,
            tc.tile_pool(name="rp", bufs=4) as rp,
            tc.tile_pool(name="tpp", bufs=3) as tpl,
            tc.tile_pool(name="hp", bufs=3) as hp,
            tc.tile_pool(name="yp", bufs=3) as yp,
            tc.tile_pool(name="ps1", bufs=2, space="PSUM") as pp1,
            tc.tile_pool(name="ps2", bufs=2, space="PSUM") as pp2,
            tc.tile_pool(name="pst", bufs=2, space="PSUM") as ppt,
        ):
            # only identf (needed by the first dgall) precedes the first xgE
            # load on the sync queue; small consts go via the idle gpsimd
            # queue and the big weights via the scalar queue.
            identf = constp.tile([128, 128], F32)
            nc.sync.dma_start(out=identf[:], in_=ident_d[:])
            rmsw2 = constp.tile([128, 2], F32)
            nc.gpsimd.dma_start(out=rmsw2[:], in_=rmsw_d[:])
            b1s = constp.tile([128, 8], F32)
            nc.gpsimd.dma_start(out=b1s[:], in_=b1_d[:])
            b2r = constp.tile([128, C], F32)
            nc.gpsimd.dma_start(out=b2r[:], in_=b2_d[:])
            w1b = constp.tile([128, 2, WH], BF16)
            nc.scalar.dma_start(out=w1b[:], in_=w1_d[:])
            w2b = constp.tile([128, 8, C], BF16)
            nc.scalar.dma_start(out=w2b[:], in_=w2_d[:])

            def emit_norm(g0, gs):
                """Load a group and compute its rstd diag blocks (DMA+vector).

                Emitted one group AHEAD of emit_compute so the next group's
                diag matrices are ready before the PE finishes the previous
                group -- otherwise these vector ops queue behind the previous
                group's PSUM evacuations and the PE stalls (and drops out of
                its max p-state, which needs ~3us of continuous busy time).
                """
                xgE = xp.tile([128, G, C], BF16, tag="xgE")
                nc.sync.dma_start(out=xgE[:, :gs, :], in_=xg_pc[:, g0:g0 + gs, :])
                sq = xp.tile([128, G, C], BF16, tag="sq")
                nc.vector.tensor_mul(sq[:, :gs, :], xgE[:, :gs, :], xgE[:, :gs, :])
                ms = rp.tile([128, G], F32, tag="ms")
                nc.vector.reduce_sum(ms[:, :gs], sq[:, :gs, :], axis=mybir.AxisListType.X)

                # rstd = rsqrt(ms/C + eps), DVE-only: quake bit-hack +
                # 1 Newton step (max rel err ~1.8e-3, inside bf16 noise).
                # C0 - (i >> 1) is built as ((i >> 1) ^ ~0) + (C0 + 1).
                msc = rp.tile([128, G], F32, tag="msc")
                nc.vector.tensor_scalar(
                    out=msc[:, :gs], in0=ms[:, :gs], scalar1=1.0 / C,
                    scalar2=RMS_EPS, op0=AluOpType.mult, op1=AluOpType.add,
                )
                y0i = rp.tile([128, G], I32, tag="y0i")
                nc.vector.tensor_scalar(
                    out=y0i[:, :gs], in0=msc[:, :gs].bitcast(I32),
                    scalar1=1, scalar2=-1,
                    op0=AluOpType.logical_shift_right, op1=AluOpType.bitwise_xor,
                )
                nc.vector.tensor_scalar(
                    out=y0i[:, :gs], in0=y0i[:, :gs],
                    scalar1=0x5F3759DF + 1, scalar2=None, op0=AluOpType.add,
                )
                y0 = y0i[:, :gs].bitcast(F32)
                nb2 = rp.tile([128, G], F32, tag="nb2")
                nc.vector.tensor_mul(nb2[:, :gs], y0, y0)
                nc.vector.tensor_mul(nb2[:, :gs], nb2[:, :gs], msc[:, :gs])
                nc.vector.tensor_scalar(
                    out=nb2[:, :gs], in0=nb2[:, :gs], scalar1=-0.5,
                    scalar2=1.5, op0=AluOpType.mult, op1=AluOpType.add,
                )
                rsq = rp.tile([128, G], F32, tag="rsq")
                nc.vector.tensor_mul(rsq[:, :gs], nb2[:, :gs], y0)
                dgall = rp.tile([128, G, 128], BF16, tag="dgall")
                nc.vector.tensor_tensor(
                    out=dgall[:, :gs, :],
                    in0=identf[:].rearrange("p (o c) -> p o c", o=1)
                        .to_broadcast([128, gs, 128]),
                    in1=rsq[:, :gs].rearrange("p (g o) -> p g o", o=1)
                        .to_broadcast([128, gs, 128]),
                    op=AluOpType.mult,
                )
                return xgE, dgall

            def emit_compute(g0, gs, xgE, dgall):
                """Transposes + MLP + store for one group (PE/scalar/vector)."""
                n = gs * 128
                # transpose each 128-row block on the PE, fusing the
                # per-token rstd scale (diag matmul).  All 2*gs transposes of
                # the group land in ONE [128, 1024] PSUM tile (each matmul
                # writes a 128-col slice, none crossing a 512-f32 bank), so
                # the PE runs them back-to-back with no buffer-rotation
                # stalls, and a single strided vector copy evacuates them.
                xgT = tpl.tile([128, 2, G * 128], BF16, tag="xgT")
                tpb = ppt.tile([128, G * 2 * 128], F32, tag="tpb")
                for j in range(gs):
                    for cc in range(2):
                        nc.tensor.matmul(
                            tpb[:, (j * 2 + cc) * 128:(j * 2 + cc + 1) * 128],
                            lhsT=xgE[:, j, cc * 128:(cc + 1) * 128],
                            rhs=dgall[:, j, :], start=True, stop=True,
                        )
                tv = tpb[:, :gs * 256].rearrange(
                    "p (j cc c) -> p cc j c", cc=2, c=128)
                xv = xgT[:, :, :n].rearrange("p cc (j c) -> p cc j c", c=128)
                if apply_rmsw:
                    for cc in range(2):
                        nc.vector.tensor_scalar(
                            out=xv[:, cc], in0=tv[:, cc],
                            scalar1=rmsw2[:, cc:cc + 1], scalar2=None,
                            op0=AluOpType.mult,
                        )
                else:
                    nc.vector.tensor_copy(xv, tv)

                hT = hp.tile([128, 8, G * 128], BF16, tag="hT")
                for wc in range(8):
                    hps = pp1.tile([128, G * 128], F32, tag="hps")
                    for h0 in range(0, n, 256):
                        hn = min(256, n - h0)
                        for cc in range(2):
                            nc.tensor.matmul(
                                hps[:, h0:h0 + hn],
                                lhsT=w1b[:, cc, wc * 128:(wc + 1) * 128],
                                rhs=xgT[:, cc, h0:h0 + hn],
                                start=(cc == 0), stop=(cc == 1),
                            )
                    nc.scalar.activation(
                        hT[:, wc, :n], hps[:, :n], ACT_GELU,
                        bias=b1s[:, wc:wc + 1],
                    )

                yE = yp.tile([128, G, C], BF16, tag="yE")
                for j in range(gs):
                    yps = pp2.tile([128, C], F32, tag="yps")
                    for wc in range(8):
                        nc.tensor.matmul(
                            yps[:],
                            lhsT=hT[:, wc, j * 128:(j + 1) * 128],
                            rhs=w2b[:, wc, :],
                            start=(wc == 0), stop=(wc == 7),
                        )
                    nc.vector.tensor_add(yE[:, j, :], yps[:], b2r[:])
                nc.sync.dma_start(out=y_pc[:, g0:g0 + gs, :], in_=yE[:, :gs, :])

            # software pipeline: norms run two groups ahead of compute so
            # the next diag blocks are always ready when the PE gets there
            ng = len(groups)
            handles = [emit_norm(*groups[i]) for i in range(min(2, ng))]
            for i in range(ng):
                if i + 2 < ng:
                    handles.append(emit_norm(*groups[i + 2]))
                emit_compute(*groups[i], *handles.pop(0))

    nc.compile()
    return nc


def _prep(x, rms_weight, W1, b1, W2, b2, expert_ids):
    import ml_dtypes

    Bb = ml_dtypes.bfloat16
    xbf = np.ascontiguousarray(
        np.asarray(x, dtype=np.float32).reshape(NTOK, C)
    ).astype(Bb)
    rmsw = np.asarray(rms_weight, dtype=np.float32).reshape(C)
    W1 = np.asarray(W1, dtype=np.float32)
    b1 = np.asarray(b1, dtype=np.float32)
    W2 = np.asarray(W2, dtype=np.float32)
    b2 = np.asarray(b2, dtype=np.float32)
    ids = np.asarray(expert_ids).reshape(-1).astype(np.int64)  # slot s -> e

    order = np.argsort(ids, kind="stable")
    counts = np.bincount(ids, minlength=E)
    bounds = np.concatenate([[0], np.cumsum(counts)])

    # dedup: both k-slots of a token on the same expert share one row
    slot_lists = []
    row_maps = []
    tok_lists = []
    ucounts = []
    for e in range(E):
        sl = order[bounds[e]:bounds[e + 1]]
        toks = sl // K
        keep = np.ones(len(sl), dtype=bool)
        if len(sl) > 1:
            keep[1:] = toks[1:] != toks[:-1]
        slot_lists.append(sl)
        row_maps.append(np.cumsum(keep) - 1)
        tok_lists.append(toks[keep])
        ucounts.append(int(keep.sum()))

    nb = max(1, math.ceil(max(ucounts) / 128))
    cap = nb * 128

    rmsw2 = np.ascontiguousarray(rmsw.reshape(2, 128).T)          # [128,2]
    identf = np.eye(128, dtype=np.float32)
    apply_rmsw = bool(not np.allclose(rmsw, 1.0))

    in_maps = []
    for e in range(E):
        toks = tok_lists[e]
        xg = np.zeros((cap, C), dtype=Bb)
        xg[:len(toks)] = xbf[toks]
        in_maps.append({
            "xg": xg,
            "w1": np.ascontiguousarray(
                W1[e].reshape(2, 128, WH).transpose(1, 0, 2)).astype(Bb),
            "w2": np.ascontiguousarray(
                W2[e].reshape(8, 128, C).transpose(1, 0, 2)).astype(Bb),
            "b1s": np.ascontiguousarray(b1[e].reshape(8, 128).T),
            "b2r": np.ascontiguousarray(
                np.broadcast_to(b2[e], (128, C))),
            "rmsw2": rmsw2,
            "identf": identf,
        })
    return in_maps, slot_lists, row_maps, nb, apply_rmsw


def run(inputs, trace=False, tmpdir=None):
    in_maps, slot_lists, row_maps, nb, apply_rmsw = _prep(**inputs)
    key = (nb, apply_rmsw)
    if key not in _CACHE:
        _CACHE[key] = _build(nb, apply_rmsw)
    nc = _CACHE[key]
    kw = {}
    if trace:
        kw = dict(trace=True, tmpdir=tmpdir)
    res = bass_utils.run_bass_kernel_spmd(nc, in_maps, core_ids=list(range(E)), **kw)
    out = np.empty((NSLOT, C), dtype=np.float32)
    for e in range(E):
        out[slot_lists[e]] = res.results[e]["y"][row_maps[e]]
    return out.reshape(B, T, K, C), res


def kernel(**inputs) -> np.ndarray:
    out, _ = run(inputs)
    return out


# revision 46
# speedup vs baseline: 1.0050x; 1.0050x over previous
"""Trainium2 Bass kernel for nn_ExpertsMLPBlock (MoE routing).

Problem (hardcoded):
  x          [8, 4096, 256] f32
  rms_weight [256]          f32
  W1         [8, 256, 1024] f32
  b1         [8, 1024]      f32
  W2         [8, 1024, 256] f32
  b2         [8, 256]       f32
  expert_ids [8, 4096, 2]   int   (values 0..7)
  out        [8, 4096, 2, 256] f32

Sharding: EXPERT-parallel across the 8 NeuronCores (the spec's suggested
"shard W1/b1/W2/b2 along the expert axis and all-to-all tokens by
expert_id").  The all-to-all happens at shard/unshard time on the host:
core e receives exactly the token rows routed to expert e (deduplicated
when both k-slots of a token pick the same expert, padded to a common
block count so all cores run the same SPMD program), plus only expert
e's weights.  The core does all the value math on device:

  rstd = rsqrt(mean(x^2) + eps)            (vector square+reduce+bit-hack
                                            rsqrt with one Newton step)
  xn^T = (x_blk @ diag(rstd))^T [* rms_w]  (PE transpose fused with the
                                            rstd column scale; all-ones
                                            rms_weight multiply skipped)
  h^T  = gelu(W1^T xn^T + b1)              (PE matmul, scalar gelu)
  y    = h W2 + b2                         (PE matmul, vector bias add)

y rows come back rank-ordered per expert (bf16); the host places them
into the full [B,T,K,C] f32 output during unsharding.

Engine budget per core: PE is the bottleneck (~125us of matmul incl.
transposes, ~97% occupied in steady state); scalar runs gelu, vector
runs the norm chain, the single batched transpose evacuation per group,
and the bias adds.  All 2*gs transposes of a group land in one
[128,1024] PSUM tile so the PE never stalls on evacuation buffers.
(Tried and regressed: XBAR dma-transpose via a DRAM bounce, gpsimd ALU
offload -- Pool cannot touch PSUM and is Q7-emulated -- and y-stores on
the sync queue, which block the xg prefetches.)
"""

import math

import numpy as np

import concourse.bacc as bacc
import concourse.bass as bass
import concourse.mybir as mybir
from concourse import bass_utils
from concourse.tile import TileContext
from concourse.alu_op_type import AluOpType

F32 = mybir.dt.float32
BF16 = mybir.dt.bfloat16
I32 = mybir.dt.int32

B, T, C, WH, E, K = 8, 4096, 256, 1024, 8, 2
NTOK = B * T            # 32768 tokens total
NSLOT = NTOK * K        # 65536 (token, k) slots
G = 4                   # blocks (of 128 rows) per pipeline group
RMS_EPS = 1.1920928955078125e-07
ACT_GELU = mybir.ActivationFunctionType.Gelu
ACT_COPY = mybir.ActivationFunctionType.Copy

_CACHE = {}


def _build(nb, apply_rmsw=False):
    """Per-core program: one expert's 2-layer MLP over nb*128 routed rows."""
    cap = nb * 128
    nc = bacc.Bacc("TRN2", target_bir_lowering=False, debug=False, num_devices=8)

    xg_d = nc.dram_tensor("xg", [cap, C], BF16, kind="ExternalInput")
    w1_d = nc.dram_tensor("w1", [128, 2, WH], BF16, kind="ExternalInput")
    w2_d = nc.dram_tensor("w2", [128, 8, C], BF16, kind="ExternalInput")
    b1_d = nc.dram_tensor("b1s", [128, 8], F32, kind="ExternalInput")
    b2_d = nc.dram_tensor("b2r", [128, C], F32, kind="ExternalInput")
    rmsw_d = nc.dram_tensor("rmsw2", [128, 2], F32, kind="ExternalInput")
    ident_d = nc.dram_tensor("identf", [128, 128], F32, kind="ExternalInput")
    y_d = nc.dram_tensor("y", [cap, C], BF16, kind="ExternalOutput")

    xg_pc = xg_d.ap().rearrange("(blk p) c -> p blk c", p=128)
    y_pc = y_d.ap().rearrange("(blk p) c -> p blk c", p=128)

    # ramp-up/ramp-down group schedule: small groups at both ends shorten
    # the pipeline fill (first matmul sooner) and drain (last store smaller)
    sizes = []
    rem = nb
    for s in (1, 2):
        if rem > s + 3:
            sizes.append(s)
            rem -= s
    tailsz = [1, 1] if rem > G + 2 else []
    rem -= len(tailsz)
    sizes += [G] * (rem // G)
    if rem % G:
        sizes.append(rem % G)
    sizes += tailsz
    groups = []
    g0 = 0
    for gs in sizes:
        groups.append((g0, gs))
        g0 += gs
    assert g0 == nb

    with TileContext(nc) as tc:
        with (
            tc.tile_pool(name="const", bufs=1) as constp,
            tc.tile_pool(name="xp", bufs=4) as xp,
            tc.tile_pool(name="rp", bufs=4) as rp,
            tc.tile_pool(name="tpp", bufs=3) as tpl,
            tc.tile_pool(name="hp", bufs=3) as hp,
            tc.tile_pool(name="yp", bufs=3) as yp,
            tc.tile_pool(name="ps1", bufs=2, space="PSUM") as pp1,
            tc.tile_pool(name="ps2", bufs=2, space="PSUM") as pp2,
            tc.tile_pool(name="pst", bufs=2, space="PSUM") as ppt,
        ):
            # only identf (needed by the first dgall) precedes the first xgE
            # load on the sync queue; small consts go via the idle gpsimd
            # queue and the big weights via the scalar queue.
            identf = constp.tile([128, 128], F32)
            nc.sync.dma_start(out=identf[:], in_=ident_d[:])
            rmsw2 = constp.tile([128, 2], F32)
            nc.gpsimd.dma_start(out=rmsw2[:], in_=rmsw_d[:])
            b1s = constp.tile([128, 8], F32)
            nc.gpsimd.dma_start(out=b1s[:], in_=b1_d[:])
            b2r = constp.tile([128, C], F32)
            nc.gpsimd.dma_start(out=b2r[:], in_=b2_d[:])
            w1b = constp.tile([128, 2, WH], BF16)
            nc.scalar.dma_start(out=w1b[:], in_=w1_d[:])
            w2b = constp.tile([128, 8, C], BF16)
            nc.scalar.dma_start(out=w2b[:], in_=w2_d[:])

            def emit_norm(g0, gs):
                """Load a group and compute its rstd diag blocks (DMA+vector).

                Emitted one group AHEAD of emit_compute so the next group's
                diag matrices are ready before the PE finishes the previous
                group -- otherwise these vector ops queue behind the previous
                group's PSUM evacuations and the PE stalls (and drops out of
                its max p-state, which needs ~3us of continuous busy time).
                """
                xgE = xp.tile([128, G, C], BF16, tag="xgE")
                nc.sync.dma_start(out=xgE[:, :gs, :], in_=xg_pc[:, g0:g0 + gs, :])
                sq = xp.tile([128, G, C], BF16, tag="sq")
                nc.vector.tensor_mul(sq[:, :gs, :], xgE[:, :gs, :], xgE[:, :gs, :])
                ms = rp.tile([128, G], F32, tag="ms")
                nc.vector.reduce_sum(ms[:, :gs], sq[:, :gs, :], axis=mybir.AxisListType.X)

                # rstd = rsqrt(ms/C + eps), DVE-only: quake bit-hack +
                # 1 Newton step (max rel err ~1.8e-3, inside bf16 noise).
                # C0 - (i >> 1) is built as ((i >> 1) ^ ~0) + (C0 + 1).
                msc = rp.tile([128, G], F32, tag="msc")
                nc.vector.tensor_scalar(
                    out=msc[:, :gs], in0=ms[:, :gs], scalar1=1.0 / C,
                    scalar2=RMS_EPS, op0=AluOpType.mult, op1=AluOpType.add,
                )
                y0i = rp.tile([128, G], I32, tag="y0i")
                nc.vector.tensor_scalar(
                    out=y0i[:, :gs], in0=msc[:, :gs].bitcast(I32),
                    scalar1=1, scalar2=-1,
                    op0=AluOpType.logical_shift_right, op1=AluOpType.bitwise_xor,
                )
                nc.vector.tensor_scalar(
                    out=y0i[:, :gs], in0=y0i[:, :gs],
                    scalar1=0x5F3759DF + 1, scalar2=None, op0=AluOpType.add,
                )
                y0 = y0i[:, :gs].bitcast(F32)
                nb2 = rp.tile([128, G], F32, tag="nb2")
                nc.vector.tensor_mul(nb2[:, :gs], y0, y0)
                nc.vector.tensor_mul(nb2[:, :gs], nb2[:, :gs], msc[:, :gs])
                nc.vector.tensor_scalar(
                    out=nb2[:, :gs], in0=nb2[:, :gs], scalar1=-0.5,
                    scalar2=1.5, op0=AluOpType.mult, op1=AluOpType.add,
                )
                rsq = rp.tile([128, G], F32, tag="rsq")
                nc.vector.tensor_mul(rsq[:, :gs], nb2[:, :gs], y0)
                dgall = rp.tile([128, G, 128], BF16, tag="dgall")
                nc.vector.tensor_tensor(
                    out=dgall[:, :gs, :],
                    in0=identf[:].rearrange("p (o c) -> p o c", o=1)
                        .to_broadcast([128, gs, 128]),
                    in1=rsq[:, :gs].rearrange("p (g o) -> p g o", o=1)
                        .to_broadcast([128, gs, 128]),
                    op=AluOpType.mult,
                )
                return xgE, dgall

            def emit_compute(g0, gs, xgE, dgall):
                """Transposes + MLP + store for one group (PE/scalar/vector)."""
                n = gs * 128
                # transpose each 128-row block on the PE, fusing the
                # per-token rstd scale (diag matmul).  All 2*gs transposes of
                # the group land in ONE [128, 1024] PSUM tile (each matmul
                # writes a 128-col slice, none crossing a 512-f32 bank), so
                # the PE runs them back-to-back with no buffer-rotation
                # stalls, and a single strided vector copy evacuates them.
                xgT = tpl.tile([128, 2, G * 128], BF16, tag="xgT")
                tpb = ppt.tile([128, G * 2 * 128], F32, tag="tpb")
                for j in range(gs):
                    for cc in range(2):
                        nc.tensor.matmul(
                            tpb[:, (j * 2 + cc) * 128:(j * 2 + cc + 1) * 128],
                            lhsT=xgE[:, j, cc * 128:(cc + 1) * 128],
                            rhs=dgall[:, j, :], start=True, stop=True,
                        )
                tv = tpb[:, :gs * 256].rearrange(
                    "p (j cc c) -> p cc j c", cc=2, c=128)
                xv = xgT[:, :, :n].rearrange("p cc (j c) -> p cc j c", c=128)
                if apply_rmsw:
                    for cc in range(2):
                        nc.vector.tensor_scalar(
                            out=xv[:, cc], in0=tv[:, cc],
                            scalar1=rmsw2[:, cc:cc + 1], scalar2=None,
                            op0=AluOpType.mult,
                        )
                else:
                    nc.vector.tensor_copy(xv, tv)

                hT = hp.tile([128, 8, G * 128], BF16, tag="hT")
                for wc in range(8):
                    hps = pp1.tile([128, G * 128], F32, tag="hps")
                    for h0 in range(0, n, 256):
                        hn = min(256, n - h0)
                        for cc in range(2):
                            nc.tensor.matmul(
                                hps[:, h0:h0 + hn],
                                lhsT=w1b[:, cc, wc * 128:(wc + 1) * 128],
                                rhs=xgT[:, cc, h0:h0 + hn],
                                start=(cc == 0), stop=(cc == 1),
                            )
                    nc.scalar.activation(
                        hT[:, wc, :n], hps[:, :n], ACT_GELU,
                        bias=b1s[:, wc:wc + 1],
                    )

                yE = yp.tile([128, G, C], BF16, tag="yE")
                for j in range(gs):
                    yps = pp2.tile([128, C], F32, tag="yps")
                    for wc in range(8):
                        nc.tensor.matmul(
                            yps[:],
                            lhsT=hT[:, wc, j * 128:(j + 1) * 128],
                            rhs=w2b[:, wc, :],
                            start=(wc == 0), stop=(wc == 7),
                        )
                    nc.vector.tensor_add(yE[:, j, :], yps[:], b2r[:])
                nc.scalar.dma_start(out=y_pc[:, g0:g0 + gs, :], in_=yE[:, :gs, :])

            # software pipeline: norms run two groups ahead of compute so
            # the next diag blocks are always ready when the PE gets there
            ng = len(groups)
            handles = [emit_norm(*groups[i]) for i in range(min(2, ng))]
            for i in range(ng):
                if i + 2 < ng:
                    handles.append(emit_norm(*groups[i + 2]))
                emit_compute(*groups[i], *handles.pop(0))

    nc.compile()
    return nc


def _prep(x, rms_weight, W1, b1, W2, b2, expert_ids):
    import ml_dtypes

    Bb = ml_dtypes.bfloat16
    xbf = np.ascontiguousarray(
        np.asarray(x, dtype=np.float32).reshape(NTOK, C)
    ).astype(Bb)
    rmsw = np.asarray(rms_weight, dtype=np.float32).reshape(C)
    W1 = np.asarray(W1, dtype=np.float32)
    b1 = np.asarray(b1, dtype=np.float32)
    W2 = np.asarray(W2, dtype=np.float32)
    b2 = np.asarray(b2, dtype=np.float32)
    ids = np.asarray(expert_ids).reshape(-1).astype(np.int64)  # slot s -> e

    order = np.argsort(ids, kind="stable")
    counts = np.bincount(ids, minlength=E)
    bounds = np.concatenate([[0], np.cumsum(counts)])

    # dedup: both k-slots of a token on the same expert share one row
    slot_lists = []
    row_maps = []
    tok_lists = []
    ucounts = []
    for e in range(E):
        sl = order[bounds[e]:bounds[e + 1]]
        toks = sl // K
        keep = np.ones(len(sl), dtype=bool)
        if len(sl) > 1:
            keep[1:] = toks[1:] != toks[:-1]
        slot_lists.append(sl)
        row_maps.append(np.cumsum(keep) - 1)
        tok_lists.append(toks[keep])
        ucounts.append(int(keep.sum()))

    nb = max(1, math.ceil(max(ucounts) / 128))
    cap = nb * 128

    rmsw2 = np.ascontiguousarray(rmsw.reshape(2, 128).T)          # [128,2]
    identf = np.eye(128, dtype=np.float32)
    apply_rmsw = bool(not np.allclose(rmsw, 1.0))

    in_maps = []
    for e in range(E):
        toks = tok_lists[e]
        xg = np.zeros((cap, C), dtype=Bb)
        xg[:len(toks)] = xbf[toks]
        in_maps.append({
            "xg": xg,
            "w1": np.ascontiguousarray(
                W1[e].reshape(2, 128, WH).transpose(1, 0, 2)).astype(Bb),
            "w2": np.ascontiguousarray(
                W2[e].reshape(8, 128, C).transpose(1, 0, 2)).astype(Bb),
            "b1s": np.ascontiguousarray(b1[e].reshape(8, 128).T),
            "b2r": np.ascontiguousarray(
                np.broadcast_to(b2[e], (128, C))),
            "rmsw2": rmsw2,
            "identf": identf,
        })
    return in_maps, slot_lists, row_maps, nb, apply_rmsw


def run(inputs, trace=False, tmpdir=None):
    in_maps, slot_lists, row_maps, nb, apply_rmsw = _prep(**inputs)
    key = (nb, apply_rmsw)
    if key not in _CACHE:
        _CACHE[key] = _build(nb, apply_rmsw)
    nc = _CACHE[key]
    kw = {}
    if trace:
        kw = dict(trace=True, tmpdir=tmpdir)
    res = bass_utils.run_bass_kernel_spmd(nc, in_maps, core_ids=list(range(E)), **kw)
    out = np.empty((NSLOT, C), dtype=np.float32)
    for e in range(E):
        out[slot_lists[e]] = res.results[e]["y"][row_maps[e]]
    return out.reshape(B, T, K, C), res


def kernel(**inputs) -> np.ndarray:
    out, _ = run(inputs)
    return out


# revision 49
# speedup vs baseline: 1.0051x; 1.0001x over previous
"""Trainium2 Bass kernel for nn_ExpertsMLPBlock (MoE routing).

Problem (hardcoded):
  x          [8, 4096, 256] f32
  rms_weight [256]          f32
  W1         [8, 256, 1024] f32
  b1         [8, 1024]      f32
  W2         [8, 1024, 256] f32
  b2         [8, 256]       f32
  expert_ids [8, 4096, 2]   int   (values 0..7)
  out        [8, 4096, 2, 256] f32

Sharding: EXPERT-parallel across the 8 NeuronCores (the spec's suggested
"shard W1/b1/W2/b2 along the expert axis and all-to-all tokens by
expert_id").  The all-to-all happens at shard/unshard time on the host:
core e receives exactly the token rows routed to expert e (deduplicated
when both k-slots of a token pick the same expert, padded to a common
block count so all cores run the same SPMD program), plus only expert
e's weights.  The core does all the value math on device:

  rstd = rsqrt(mean(x^2) + eps)            (vector square+reduce+bit-hack
                                            rsqrt with one Newton step)
  xn^T = (x_blk @ diag(rstd))^T [* rms_w]  (PE transpose fused with the
                                            rstd column scale; all-ones
                                            rms_weight multiply skipped)
  h^T  = gelu(W1^T xn^T + b1)              (PE matmul, scalar gelu)
  y    = h W2 + b2                         (PE matmul, vector bias add)

y rows come back rank-ordered per expert (bf16); the host places them
into the full [B,T,K,C] f32 output during unsharding.

Engine budget per core: PE is the bottleneck (~140us of matmul incl.
transposes); scalar runs gelu + half the transpose evacuations, vector
runs the rest.  (An XBAR dma-transpose variant was tried and regressed:
the DRAM bounce serializes the sync queue.)  GpSimd/Pool cannot touch
PSUM and its ALU ops are Q7-emulated, so it is left idle.
"""

import math

import numpy as np

import concourse.bacc as bacc
import concourse.bass as bass
import concourse.mybir as mybir
from concourse import bass_utils
from concourse.tile import TileContext
from concourse.alu_op_type import AluOpType

F32 = mybir.dt.float32
BF16 = mybir.dt.bfloat16
I32 = mybir.dt.int32

B, T, C, WH, E, K = 8, 4096, 256, 1024, 8, 2
NTOK = B * T            # 32768 tokens total
NSLOT = NTOK * K        # 65536 (token, k) slots
G = 4                   # blocks (of 128 rows) per pipeline group
RMS_EPS = 1.1920928955078125e-07
ACT_GELU = mybir.ActivationFunctionType.Gelu
ACT_COPY = mybir.ActivationFunctionType.Copy

_CACHE = {}


def _build(nb, apply_rmsw=False):
    """Per-core program: one expert's 2-layer MLP over nb*128 routed rows."""
    cap = nb * 128
    nc = bacc.Bacc("TRN2", target_bir_lowering=False, debug=False, num_devices=8)

    xg_d = nc.dram_tensor("xg", [cap, C], BF16, kind="ExternalInput")
    w1_d = nc.dram_tensor("w1", [128, 2, WH], BF16, kind="ExternalInput")
    w2_d = nc.dram_tensor("w2", [128, 8, C], BF16, kind="ExternalInput")
    b1_d = nc.dram_tensor("b1s", [128, 8], F32, kind="ExternalInput")
    b2_d = nc.dram_tensor("b2r", [128, C], F32, kind="ExternalInput")
    rmsw_d = nc.dram_tensor("rmsw2", [128, 2], F32, kind="ExternalInput")
    ident_d = nc.dram_tensor("identf", [128, 128], F32, kind="ExternalInput")
    y_d = nc.dram_tensor("y", [cap, C], BF16, kind="ExternalOutput")

    xg_pc = xg_d.ap().rearrange("(blk p) c -> p blk c", p=128)
    y_pc = y_d.ap().rearrange("(blk p) c -> p blk c", p=128)

    # ramp-up/ramp-down group schedule: small groups at both ends shorten
    # the pipeline fill (first matmul sooner) and drain (last store smaller)
    sizes = []
    rem = nb
    for s in (1, 2):
        if rem > s + 3:
            sizes.append(s)
            rem -= s
    tailsz = [1, 1] if rem > G + 2 else []
    rem -= len(tailsz)
    sizes += [G] * (rem // G)
    if rem % G:
        sizes.append(rem % G)
    sizes += tailsz
    groups = []
    g0 = 0
    for gs in sizes:
        groups.append((g0, gs))
        g0 += gs
    assert g0 == nb

    with TileContext(nc) as tc:
        with (
            tc.tile_pool(name="const", bufs=1) as constp,
            tc.tile_pool(name="xp", bufs=3) as xp,
            tc.tile_pool(name="rp", bufs=3) as rp,
            tc.tile_pool(name="tpp", bufs=3) as tpl,
            tc.tile_pool(name="hp", bufs=3) as hp,
            tc.tile_pool(name="yp", bufs=3) as yp,
            tc.tile_pool(name="ps1", bufs=2, space="PSUM") as pp1,
            tc.tile_pool(name="ps2", bufs=2, space="PSUM") as pp2,
            tc.tile_pool(name="pst", bufs=2, space="PSUM") as ppt,
        ):
            # only identf (needed by the first dgall) precedes the first xgE
            # load on the sync queue; small consts go via the idle gpsimd
            # queue and the big weights via the scalar queue.
            identf = constp.tile([128, 128], F32)
            nc.sync.dma_start(out=identf[:], in_=ident_d[:])
            rmsw2 = constp.tile([128, 2], F32)
            nc.gpsimd.dma_start(out=rmsw2[:], in_=rmsw_d[:])
            b1s = constp.tile([128, 8], F32)
            nc.gpsimd.dma_start(out=b1s[:], in_=b1_d[:])
            b2r = constp.tile([128, C], F32)
            nc.gpsimd.dma_start(out=b2r[:], in_=b2_d[:])
            w1b = constp.tile([128, 2, WH], BF16)
            nc.scalar.dma_start(out=w1b[:], in_=w1_d[:])
            w2b = constp.tile([128, 8, C], BF16)
            nc.scalar.dma_start(out=w2b[:], in_=w2_d[:])

            def emit_norm(g0, gs):
                """Load a group and compute its rstd diag blocks (DMA+vector).

                Emitted one group AHEAD of emit_compute so the next group's
                diag matrices are ready before the PE finishes the previous
                group -- otherwise these vector ops queue behind the previous
                group's PSUM evacuations and the PE stalls (and drops out of
                its max p-state, which needs ~3us of continuous busy time).
                """
                xgE = xp.tile([128, G, C], BF16, tag="xgE")
                nc.sync.dma_start(out=xgE[:, :gs, :], in_=xg_pc[:, g0:g0 + gs, :])
                sq = xp.tile([128, G, C], BF16, tag="sq")
                nc.vector.tensor_mul(sq[:, :gs, :], xgE[:, :gs, :], xgE[:, :gs, :])
                ms = rp.tile([128, G], F32, tag="ms")
                nc.vector.reduce_sum(ms[:, :gs], sq[:, :gs, :], axis=mybir.AxisListType.X)

                # rstd = rsqrt(ms/C + eps), DVE-only: quake bit-hack +
                # 1 Newton step (max rel err ~1.8e-3, inside bf16 noise).
                # C0 - (i >> 1) is built as ((i >> 1) ^ ~0) + (C0 + 1).
                msc = rp.tile([128, G], F32, tag="msc")
                nc.vector.tensor_scalar(
                    out=msc[:, :gs], in0=ms[:, :gs], scalar1=1.0 / C,
                    scalar2=RMS_EPS, op0=AluOpType.mult, op1=AluOpType.add,
                )
                y0i = rp.tile([128, G], I32, tag="y0i")
                nc.vector.tensor_scalar(
                    out=y0i[:, :gs], in0=msc[:, :gs].bitcast(I32),
                    scalar1=1, scalar2=-1,
                    op0=AluOpType.logical_shift_right, op1=AluOpType.bitwise_xor,
                )
                nc.vector.tensor_scalar(
                    out=y0i[:, :gs], in0=y0i[:, :gs],
                    scalar1=0x5F3759DF + 1, scalar2=None, op0=AluOpType.add,
                )
                y0 = y0i[:, :gs].bitcast(F32)
                nb2 = rp.tile([128, G], F32, tag="nb2")
                nc.vector.tensor_mul(nb2[:, :gs], y0, y0)
                nc.vector.tensor_mul(nb2[:, :gs], nb2[:, :gs], msc[:, :gs])
                nc.vector.tensor_scalar(
                    out=nb2[:, :gs], in0=nb2[:, :gs], scalar1=-0.5,
                    scalar2=1.5, op0=AluOpType.mult, op1=AluOpType.add,
                )
                rsq = rp.tile([128, G], F32, tag="rsq")
                nc.vector.tensor_mul(rsq[:, :gs], nb2[:, :gs], y0)
                dgall = rp.tile([128, G, 128], BF16, tag="dgall")
                nc.vector.tensor_tensor(
                    out=dgall[:, :gs, :],
                    in0=identf[:].rearrange("p (o c) -> p o c", o=1)
                        .to_broadcast([128, gs, 128]),
                    in1=rsq[:, :gs].rearrange("p (g o) -> p g o", o=1)
                        .to_broadcast([128, gs, 128]),
                    op=AluOpType.mult,
                )
                return xgE, dgall

            def emit_compute(g0, gs, xgE, dgall):
                """Transposes + MLP + store for one group (PE/scalar/vector)."""
                n = gs * 128
                # transpose each 128-row block on the PE, fusing the
                # per-token rstd scale (diag matmul).  All 2*gs transposes of
                # the group land in ONE [128, 1024] PSUM tile (each matmul
                # writes a 128-col slice, none crossing a 512-f32 bank), so
                # the PE runs them back-to-back with no buffer-rotation
                # stalls, and a single strided vector copy evacuates them.
                xgT = tpl.tile([128, 2, G * 128], BF16, tag="xgT")
                tpb = ppt.tile([128, G * 2 * 128], F32, tag="tpb")
                for j in range(gs):
                    for cc in range(2):
                        nc.tensor.matmul(
                            tpb[:, (j * 2 + cc) * 128:(j * 2 + cc + 1) * 128],
                            lhsT=xgE[:, j, cc * 128:(cc + 1) * 128],
                            rhs=dgall[:, j, :], start=True, stop=True,
                        )
                tv = tpb[:, :gs * 256].rearrange(
                    "p (j cc c) -> p cc j c", cc=2, c=128)
                xv = xgT[:, :, :n].rearrange("p cc (j c) -> p cc j c", c=128)
                if apply_rmsw:
                    for cc in range(2):
                        nc.vector.tensor_scalar(
                            out=xv[:, cc], in0=tv[:, cc],
                            scalar1=rmsw2[:, cc:cc + 1], scalar2=None,
                            op0=AluOpType.mult,
                        )
                else:
                    nc.vector.tensor_copy(xv, tv)

                hT = hp.tile([128, 8, G * 128], BF16, tag="hT")
                for wc in range(8):
                    hps = pp1.tile([128, G * 128], F32, tag="hps")
                    for h0 in range(0, n, 256):
                        hn = min(256, n - h0)
                        for cc in range(2):
                            nc.tensor.matmul(
                                hps[:, h0:h0 + hn],
                                lhsT=w1b[:, cc, wc * 128:(wc + 1) * 128],
                                rhs=xgT[:, cc, h0:h0 + hn],
                                start=(cc == 0), stop=(cc == 1),
                            )
                    nc.scalar.activation(
                        hT[:, wc, :n], hps[:, :n], ACT_GELU,
                        bias=b1s[:, wc:wc + 1],
                    )

                yE = yp.tile([128, G, C], BF16, tag="yE")
                for j in range(gs):
                    yps = pp2.tile([128, C], F32, tag="yps")
                    for wc in range(8):
                        nc.tensor.matmul(
                            yps[:],
                            lhsT=hT[:, wc, j * 128:(j + 1) * 128],
                            rhs=w2b[:, wc, :],
                            start=(wc == 0), stop=(wc == 7),
                        )
                    nc.vector.tensor_add(yE[:, j, :], yps[:], b2r[:])
                nc.gpsimd.dma_start(out=y_pc[:, g0:g0 + gs, :], in_=yE[:, :gs, :])

            # software pipeline: norm(i+1) is emitted before compute(i)
            handles = emit_norm(*groups[0])
            for i in range(len(groups)):
                nxt = emit_norm(*groups[i + 1]) if i + 1 < len(groups) else None
                emit_compute(*groups[i], *handles)
                handles = nxt

    nc.compile()
    return nc


def _prep(x, rms_weight, W1, b1, W2, b2, expert_ids):
    import ml_dtypes

    Bb = ml_dtypes.bfloat16
    xbf = np.ascontiguousarray(
        np.asarray(x, dtype=np.float32).reshape(NTOK, C)
    ).astype(Bb)
    rmsw = np.asarray(rms_weight, dtype=np.float32).reshape(C)
    W1 = np.asarray(W1, dtype=np.float32)
    b1 = np.asarray(b1, dtype=np.float32)
    W2 = np.asarray(W2, dtype=np.float32)
    b2 = np.asarray(b2, dtype=np.float32)
    ids = np.asarray(expert_ids).reshape(-1).astype(np.int64)  # slot s -> e

    order = np.argsort(ids, kind="stable")
    counts = np.bincount(ids, minlength=E)
    bounds = np.concatenate([[0], np.cumsum(counts)])

    # dedup: both k-slots of a token on the same expert share one row
    slot_lists = []
    row_maps = []
    tok_lists = []
    ucounts = []
    for e in range(E):
        sl = order[bounds[e]:bounds[e + 1]]
        toks = sl // K
        keep = np.ones(len(sl), dtype=bool)
        if len(sl) > 1:
            keep[1:] = toks[1:] != toks[:-1]
        slot_lists.append(sl)
        row_maps.append(np.cumsum(keep) - 1)
        tok_lists.append(toks[keep])
        ucounts.append(int(keep.sum()))

    nb = max(1, math.ceil(max(ucounts) / 128))
    cap = nb * 128

    rmsw2 = np.ascontiguousarray(rmsw.reshape(2, 128).T)          # [128,2]
    identf = np.eye(128, dtype=np.float32)
    apply_rmsw = bool(not np.allclose(rmsw, 1.0))

    in_maps = []
    for e in range(E):
        toks = tok_lists[e]
        xg = np.zeros((cap, C), dtype=Bb)
        xg[:len(toks)] = xbf[toks]
        in_maps.append({
            "xg": xg,
            "w1": np.ascontiguousarray(
                W1[e].reshape(2, 128, WH).transpose(1, 0, 2)).astype(Bb),
            "w2": np.ascontiguousarray(
                W2[e].reshape(8, 128, C).transpose(1, 0, 2)).astype(Bb),
            "b1s": np.ascontiguousarray(b1[e].reshape(8, 128).T),
            "b2r": np.ascontiguousarray(
                np.broadcast_to(b2[e], (128, C))),
            "rmsw2": rmsw2,
            "identf": identf,
        })
    return in_maps, slot_lists, row_maps, nb, apply_rmsw


def run(inputs, trace=False, tmpdir=None):
    in_maps, slot_lists, row_maps, nb, apply_rmsw = _prep(**inputs)
    key = (nb, apply_rmsw)
    if key not in _CACHE:
        _CACHE[key] = _build(nb, apply_rmsw)
    nc = _CACHE[key]
    kw = {}
    if trace:
        kw = dict(trace=True, tmpdir=tmpdir)
    res = bass_utils.run_bass_kernel_spmd(nc, in_maps, core_ids=list(range(E)), **kw)
    out = np.empty((NSLOT, C), dtype=np.float32)
    for e in range(E):
        out[slot_lists[e]] = res.results[e]["y"][row_maps[e]]
    return out.reshape(B, T, K, C), res


def kernel(**inputs) -> np.ndarray:
    out, _ = run(inputs)
    return out
